# revision 52
# baseline (speedup 1.0000x reference)
"""Trainium2 Bass kernel for a GPT-style transformer block.

Shapes: x [2, 2048, 1024], H=16 heads, D=64, MLP 4x.

Distribution over 8 NeuronCores: data-parallel over batch (cores 0-3 ->
batch 0, cores 4-7 -> batch 1) x sequence-parallel over tokens inside
each batch group. Tokens are stride-4 interleaved (core s of the group
owns global tokens s, s+4, ...), which makes the causal-attention loop
structure identical on every core (required: all 8 cores share one SPMD
program); the rank-dependent causal diagonal masks are shipped as
per-core input data. The only collectives are two 4-rank AllGathers per
group (K first, then V, so the Q projection overlaps them).

LayerNorm gains/biases are folded into the adjacent weight matrices on
the host, so on-chip LN is a plain standardization with per-partition
(per-token) scalars. LN1 is applied in feature-major (transposed) space
directly on a host-transposed copy of x, which removes the on-chip
PE transposes for the QKV input.

Precision: QKV and Wo run in fp8e4m3 with DoubleRow (2x PE throughput,
256-deep contraction per pass); global per-matrix dequant scales are
folded into the existing epilogue ops. K/Q/V and the exp'd attention
weights are fp8 as well, which halves both AllGather payloads and lets
PV contract 256 keys per pass via DoubleRow. The MLP (Wfc/Wproj) stays
bf16 — fp8 there pushes the output error to ~1.7e-2, too close to the
2e-2 gate. Accumulation, LN statistics, softmax normalization and
residuals stay fp32. Measured end-to-end relative error: ~2.5e-3.

The softmax denominator comes for free from an extra ones-column
appended to V (row 64 of the PV accumulator), so no partition-axis
reduction is needed; the reciprocal row is broadcast across partitions
on the (otherwise idle) GpSimd engine.
"""

import os
import sys

for _p in ("/opt/trn_rl_repo", "/root/.axon_site/_ro/trn_rl_repo"):
    if os.path.isdir(_p) and _p not in sys.path:
        sys.path.insert(0, _p)

import numpy as np
import ml_dtypes

import concourse.bass as bass
import concourse.bacc as bacc
import concourse.mybir as mybir
import concourse.tile as tile
from concourse.bass_utils import run_bass_kernel_spmd

F32 = mybir.dt.float32
F32R = mybir.dt.float32r
BF16 = mybir.dt.bfloat16
F8 = mybir.dt.float8e4
DR = mybir.MatmulPerfMode.DoubleRow
AF = mybir.ActivationFunctionType
OP = mybir.AluOpType
BF16_NP = ml_dtypes.bfloat16
F8_NP = ml_dtypes.float8_e4m3

B, T, C = 2, 2048, 1024
H, D = 16, 64
FF = 4 * C
EPS = 1e-5
P = 128
CH = C // P        # 8 chunks of the channel dim
NBQ = 4            # local 128-token blocks per core (512 tokens)
NR = 4             # seq ranks per batch group
FCH = FF // P      # 32 chunks of the FF dim
HP = H // 2        # head pairs

TRACE = False           # set by test harness for profiling
LEVEL = 5               # phase bisection: 1=QKV 2=+AG 3=+attn 4=+Wo 5=full
REPS = 1                # timing: emit the whole block N times, serialized via x
LAST_RESULTS = None     # BassKernelResults of the last run

_CACHE = {}


def _ln_stats(nc, pool, src, tag):
    """Phase 1 of LN: per-token sum and sum-of-squares of `src` (both ACT)."""
    s1 = pool.tile([P, 1], F32, name=f"ln_s1_{tag}", tag=f"ln_s1_{tag}")
    ssq = pool.tile([P, 1], F32, name=f"ln_ssq_{tag}", tag=f"ln_ssq_{tag}")
    sqs = pool.tile([P, C], BF16, name=f"ln_sqs_{tag}", tag="ln_sqs", bufs=1)
    nc.vector.reduce_sum(s1[:, :], src, axis=mybir.AxisListType.X)
    nc.scalar.activation(sqs[:, :], src, AF.Square, accum_out=ssq[:, :])
    return s1, ssq


def _ln_scalars(nc, pool, s1, ssq, tag):
    """Turn (sum, sumsq) into per-token (mean, rstd) columns."""
    mean = pool.tile([P, 1], F32, name=f"ln_mean_{tag}", tag=f"ln_mean_{tag}")
    var = pool.tile([P, 1], F32, name=f"ln_var_{tag}", tag="ln_var")
    m2 = pool.tile([P, 1], F32, name=f"ln_m2_{tag}", tag="ln_m2")
    std = pool.tile([P, 1], F32, name=f"ln_std_{tag}", tag="ln_std")
    rstd = pool.tile([P, 1], F32, name=f"ln_rstd_{tag}", tag=f"ln_rstd_{tag}")
    nc.vector.tensor_scalar_mul(mean[:, :], s1[:, :], 1.0 / C)
    nc.vector.tensor_mul(m2[:, :], mean[:, :], mean[:, :])
    nc.vector.tensor_scalar(var[:, :], ssq[:, :], 1.0 / C, EPS, OP.mult, OP.add)
    nc.vector.tensor_sub(var[:, :], var[:, :], m2[:, :])
    nc.scalar.activation(std[:, :], var[:, :], AF.Sqrt)
    nc.vector.reciprocal(rstd[:, :], std[:, :])
    return mean, rstd


def _ln_finalize(nc, pool, src, z_bf, s1, ssq, tag):
    """Phase 2 of LN: (x-mean)*rstd -> z_bf on the idle Pool engine."""
    mean, rstd = _ln_scalars(nc, pool, s1, ssq, tag)
    nc.vector.tensor_scalar(
        z_bf, src, mean[:, :], rstd[:, :], OP.subtract, OP.mult
    )


def _build(level=5, reps=1, sim=False, zb=False):
    """zb=True specializes away the ones-matmul bias injections (all
    bias vectors zero — detected from the actual inputs by kernel())."""
    if (level, reps, sim, zb) in _CACHE:
        return _CACHE[(level, reps, sim, zb)]

    nc = bacc.Bacc(
        "TRN2", target_bir_lowering=False, debug=False,
        num_devices=1 if sim else 8,
    )

    # ---- kernel I/O (per core) ----
    x_in = nc.dram_tensor("x_c", [NBQ, P, C], F32, kind="ExternalInput").ap()
    xT_in = nc.dram_tensor("xT_c", [CH, P, 512], BF16, kind="ExternalInput").ap()
    wqkv_in = nc.dram_tensor("wqkv", [CH, P, 3 * C], F8, kind="ExternalInput").ap()
    wo_in = nc.dram_tensor("wo", [CH, P, C], F8, kind="ExternalInput").ap()
    wfc_in = nc.dram_tensor("wfc", [CH, P, FF], BF16, kind="ExternalInput").ap()
    wproj_in = nc.dram_tensor("wproj", [FCH, P, C], BF16, kind="ExternalInput").ap()
    bqk_in = nc.dram_tensor("bqk", [2 * CH, P], F32, kind="ExternalInput").ap()
    bv_in = nc.dram_tensor("bv", [1, C], BF16, kind="ExternalInput").ap()
    bo_in = nc.dram_tensor("bo_r", [1, C], BF16, kind="ExternalInput").ap()
    bfc_in = nc.dram_tensor("bfc_r", [FCH, P], F32, kind="ExternalInput").ap()
    bproj_in = nc.dram_tensor("bproj_r", [1, C], BF16, kind="ExternalInput").ap()
    ident_in = nc.dram_tensor("ident", [P, P], BF16, kind="ExternalInput").ap()
    scales_in = nc.dram_tensor("scales", [P, 2], F32, kind="ExternalInput").ap()
    ones_in = nc.dram_tensor("ones_r", [1, P], BF16, kind="ExternalInput").ap()
    masks_in = nc.dram_tensor("masks", [NR, P, P], F8, kind="ExternalInput").ap()
    out_dram = nc.dram_tensor("out_c", [NBQ, P, C], F32, kind="ExternalOutput").ap()

    KCOLS = CH * 512          # 4096 bf16 cols for K^T in the AG payload
    VCOLS = NBQ * (C + H)     # 4*1040 cols for aug-V in the AG payload

    with tile.TileContext(nc) as tc:
        dramp = tc.alloc_tile_pool(name="dram", bufs=1, space="DRAM")
        rep_io = [
            dramp.tile([NBQ, P, C], F32, name=f"rep_io_{i}")
            for i in range(reps - 1)
        ]

        for rep in range(reps):
            sfx = f"_{rep}" if reps > 1 else ""
            x_src = x_in if rep == 0 else rep_io[rep - 1]
            out_tgt = out_dram if rep == reps - 1 else rep_io[rep]
            kvin_k = dramp.tile([P, KCOLS], F8, name=f"kvink{sfx}_a")
            kvout_k = dramp.tile([NR, P, KCOLS], F8, name=f"kvoutk{sfx}_a")
            kvin_v = dramp.tile([P, VCOLS], F8, name=f"kvinv{sfx}_a")
            kvout_v = dramp.tile([NR, P, VCOLS], F8, name=f"kvoutv{sfx}_a")
            # ---------------- persistent SBUF ----------------
            persist = tc.alloc_tile_pool(name=f"persist{sfx}", bufs=1, side="left")
            ident_sb = persist.tile([P, P], BF16, name="ident_sb")
            ones_sb = persist.tile([1, P], BF16, name="ones_sb")
            masks_sb = persist.tile([P, NR, P], F8, name="masks_sb")
            bqk_sb = persist.tile([P, 2 * CH], F32, name="bqk_sb")
            bv_sb = persist.tile([1, C], BF16, name="bv_sb")
            bo_sb = persist.tile([1, C], BF16, name="bo_sb")
            bfc_sb = persist.tile([P, FCH], F32, name="bfc_sb")
            bproj_sb = persist.tile([1, C], BF16, name="bproj_sb")
            scales_sb = persist.tile([P, 2], F32, name="scales_sb")
            qT = persist.tile([P, CH, 512], F8, name="qT")
            yT = persist.tile([P, CH, 512], F8, name="yT")

            xres = tc.alloc_tile_pool(name=f"xres{sfx}", bufs=1, side="left")
            # K^T gathered from all 4 ranks: [d-part, head-pair chunk, rank, tok]
            attnspan = tc.alloc_tile_pool(name=f"attnspan{sfx}", bufs=1, side="left")
            kfull = attnspan.tile([P, CH, NR, 512], F8, name="kfull")
            vfull = attnspan.tile([P, NR, NBQ, C + H], F8, name="vfull")

            # ---------------- phase 0: LN1 + QKV + AllGather ----------------
            ph0 = tc.alloc_tile_pool(name=f"ph0{sfx}", bufs=1, side="left")
            ph0w = tc.alloc_tile_pool(name=f"ph0w{sfx}", bufs=2, side="left")
            wqkv_sb = ph0.tile([P, CH, 3 * C], F8, name="wqkv_sb")
            hT = ph0.tile([P, CH, 512], F8, name="hT")
            kTc = ph0.tile([P, CH, 512], F8, name="kTc")
            vc = ph0.tile([P, NBQ, C + H], F8, name="vc")

            psQK = tc.alloc_tile_pool(name=f"psQK{sfx}", bufs=5, space="PSUM")
            psV = tc.alloc_tile_pool(name=f"psV{sfx}", bufs=2, space="PSUM")
            psT = tc.alloc_tile_pool(name=f"psT{sfx}", bufs=1, space="PSUM")

            nc.sync.dma_start(ident_sb[:, :], ident_in)
            xts, stats = [], []
            for bq in range(NBQ):
                xt = xres.tile([P, C], F32, name="xt", tag=f"xt_{bq}")
                nc.sync.dma_start(xt[:, 0:C // 2], x_src[bq][:, 0:C // 2])
                nc.sync.dma_start(xt[:, C // 2:], x_src[bq][:, C // 2:])
                if level < 5:
                    # timing-only levels: output = input, keeps the rep chain
                    # data-dependent without a rep-independent dummy write
                    nc.sync.dma_start(out_tgt[bq], xt[:, :])
                xts.append(xt)
                stats.append(_ln_stats(nc, ph0w, xt[:, :], f"l1_{bq}"))
            # LN1 in transposed space: hT[c,t] = xT[c,t]*rstd[t] - mean[t]*rstd[t]
            xT_sb = ph0.tile([P, CH, 512], BF16, name="xT_sb")
            for c in range(CH):
                nc.sync.dma_start(xT_sb[:, c, :], xT_in[c])
            mrs = []
            for bq in range(NBQ):
                mean, rstd = _ln_scalars(nc, ph0w, *stats[bq], f"l1_{bq}")
                mr = ph0w.tile([P, 2], BF16, name="mr", tag=f"mr_{bq}", bufs=1)
                nc.vector.tensor_copy(mr[:, 0:1], rstd[:, :])
                nc.vector.tensor_mul(mr[:, 1:2], mean[:, :], rstd[:, :])
                mrs.append(mr)
            ptm = psT.tile([1, 2, NBQ, P], BF16, name="ptm")
            for bq in range(NBQ):
                nc.tensor.transpose(ptm[:, 0, bq, :], mrs[bq][:, 0:1], ident_sb[:, :])
                nc.tensor.transpose(ptm[:, 1, bq, :], mrs[bq][:, 1:2], ident_sb[:, :])
            rows0 = ph0w.tile([1, NBQ * P], BF16, name="rows0", bufs=1)
            rows1 = ph0w.tile([1, NBQ * P], BF16, name="rows1", bufs=1)
            nc.vector.tensor_copy(rows0[:, :], ptm[:, 0, :, :])
            nc.vector.tensor_copy(rows1[:, :], ptm[:, 1, :, :])
            arow = ph0w.tile([P, 512], BF16, name="arow", bufs=1)
            brow = ph0w.tile([P, 512], BF16, name="brow", bufs=1)
            nc.gpsimd.partition_broadcast(arow[:, :], rows0[0:1, :])
            nc.gpsimd.partition_broadcast(brow[:, :], rows1[0:1, :])
            for c in range(CH):
                zt = ph0w.tile([P, 512], BF16, name="zt", tag="zt", bufs=2)
                nc.vector.tensor_mul(zt[:, :], xT_sb[:, c, :], arow[:, :])
                nc.vector.tensor_sub(hT[:, c, :], zt[:, :], brow[:, :])

            for c in range(CH):
                nc.sync.dma_start(wqkv_sb[:, c, :], wqkv_in[c])
            nc.sync.dma_start(bqk_sb[:, :], bqk_in.rearrange("a p -> p a"))
            nc.sync.dma_start(ones_sb[:, :], ones_in)
            for rk in range(NR):
                nc.sync.dma_start(masks_sb[:, rk, :], masks_in[rk])
            nc.sync.dma_start(bv_sb[:, :], bv_in)
            nc.sync.dma_start(bo_sb[:, :], bo_in)
            nc.sync.dma_start(bfc_sb[:, :], bfc_in.rearrange("a p -> p a"))
            nc.sync.dma_start(bproj_sb[:, :], bproj_in)
            nc.sync.dma_start(scales_sb[:, :], scales_in)

            # Q^T and K^T: [feat, tok] via lhsT=W chunk, rhs=h^T.
            # K^T first so the AllGather can launch while Q^T computes.
            def _qk_tile(ft):
                ps = psQK.tile([P, 512], F32, name="ps_qk", tag="ps_qk")
                for c2 in range(CH // 2):
                    nc.tensor.matmul(
                        ps[:, :],
                        wqkv_sb[:, 2 * c2:2 * c2 + 2, ft * P:(ft + 1) * P],
                        hT[:, 2 * c2:2 * c2 + 2, :],
                        start=(c2 == 0),
                        stop=(c2 == CH // 2 - 1),
                        perf_mode=DR,
                    )
                dest = qT[:, ft, :] if ft < CH else kTc[:, ft - CH, :]
                nc.vector.tensor_scalar(
                    dest, ps[:, :], scales_sb[:, 0:1], bqk_sb[:, ft:ft + 1],
                    OP.mult, OP.add,
                )

            for ft in range(CH, 2 * CH):
                _qk_tile(ft)

            # AllGather K then V across the 4 seq ranks of this batch group
            if level >= 2:
              groups = [[0, 1, 2, 3], [4, 5, 6, 7]]
              for q in range(4):
                  nc.sync.dma_start(
                      kvin_k[:, q * KCOLS // 4:(q + 1) * KCOLS // 4],
                      kTc[:, 2 * q:2 * q + 2, :].rearrange("p c t -> p (c t)"),
                  )
              if sim:
                  for r in range(NR):
                      nc.sync.dma_start(kvout_k[r], kvin_k[:, :])
              else:
                  nc.gpsimd.collective_compute(
                      "AllGather", OP.bypass, replica_groups=groups,
                      ins=[kvin_k.opt()], outs=[kvout_k.opt()],
                  )
            # V in [tok, feat] layout with a ones column appended per head
            # (col h*65+64) so PV also accumulates the softmax denominator.
            for bq in range(NBQ):
                for fb in range(2):
                    ps = psV.tile([P, 512], F32, name="ps_v", tag="ps_v")
                    for c2 in range(CH // 2):
                        nc.tensor.matmul(
                            ps[:, :],
                            hT[:, 2 * c2:2 * c2 + 2, bq * P:(bq + 1) * P],
                            wqkv_sb[:, 2 * c2:2 * c2 + 2,
                                    2 * C + fb * 512:2 * C + (fb + 1) * 512],
                            start=(c2 == 0),
                            stop=(zb and c2 == CH // 2 - 1),
                            perf_mode=DR,
                        )
                    if not zb:
                        nc.tensor.matmul(
                            ps[:, :],
                            ones_sb[0:1, 0:P],
                            bv_sb[0:1, fb * 512:(fb + 1) * 512],
                            start=False,
                            stop=True,
                        )
                    dst = vc[:, bq, fb * 8 * 65:(fb + 1) * 8 * 65]
                    dst = dst.rearrange("p (h x) -> p h x", x=65)[:, :, 0:64]
                    nc.vector.tensor_scalar_mul(
                        dst, ps.rearrange("p (h x) -> p h x", x=64),
                        scales_sb[:, 0:1],
                    )
            ones_lane = vc.rearrange("p b (h x) -> p b h x", x=65)[:, :, :, 64:65]
            nc.vector.memset(ones_lane, 1.0)

            if level >= 2:
              for q in range(4):
                  nc.sync.dma_start(
                      kvin_v[:, q * VCOLS // 4:(q + 1) * VCOLS // 4],
                      vc[:, q, :],
                  )
              if sim:
                  for r in range(NR):
                      nc.sync.dma_start(kvout_v[r], kvin_v[:, :])
              else:
                  nc.gpsimd.collective_compute(
                      "AllGather", OP.bypass, replica_groups=groups,
                      ins=[kvin_v.opt()], outs=[kvout_v.opt()],
                  )
            for ft in range(CH):
                _qk_tile(ft)
            if level >= 2:
              # readback ordered by first consumer: the attention loop is
              # hp-major (K) and bk-minor (V), so pull K per head-pair chunk
              # across all ranks, and V per query block across all ranks
              for cg in range(0, CH, 2):
                  for r in range(NR):
                      nc.sync.dma_start(
                          kfull[:, cg:cg + 2, r, :],
                          kvout_k[r].rearrange("p (c t) -> p c t", t=512)[
                              :, cg:cg + 2, :],
                      )
                      if cg < 2 * NBQ:
                          bk = cg // 2
                          nc.sync.dma_start(
                              vfull[:, r, bk, :],
                              kvout_v[r].rearrange(
                                  "p (b f) -> p b f", f=C + H)[:, bk, :],
                          )

            psT.release()
            psV.release()
            psQK.release()
            ph0w.release()
            ph0.release()

            # ---------------- attention ----------------
            x2pool = tc.alloc_tile_pool(name=f"x2pool{sfx}", bufs=1, side="right")
            x2 = x2pool.tile([P, NBQ, C], F32, name="x2")

            wfcpool = tc.alloc_tile_pool(name=f"wfcpool{sfx}", bufs=1, side="right")
            wfc_sb = wfcpool.tile([P, CH, FF], BF16, name="wfc_sb")
            if level >= 5:
                for c in range(CH):
                    for q in range(4):
                        nc.sync.dma_start(
                            wfc_sb[:, c, q * FF // 4:(q + 1) * FF // 4],
                            wfc_in[c][:, q * FF // 4:(q + 1) * FF // 4],
                        )

            wospan = tc.alloc_tile_pool(name=f"wospan{sfx}", bufs=1, side="right")
            wo_sb = wospan.tile([P, CH, C], F8, name="wo_sb")
            if level >= 4:
                for c in range(CH):
                    nc.sync.dma_start(wo_sb[:, c, :], wo_in[c])

            att = tc.alloc_tile_pool(name=f"att{sfx}", bufs=1, side="right")
            psS = tc.alloc_tile_pool(name=f"psS{sfx}", bufs=3, space="PSUM")
            psY = tc.alloc_tile_pool(name=f"psY{sfx}", bufs=1, space="PSUM")

            for hp in range(HP if level >= 3 else 0):
                psy = [
                    psY.tile([65, 512], F32, name=f"psy{sub}_{hp}", tag=f"psy{sub}")
                    for sub in range(2)
                ]
                for bk in range(NBQ):
                    qo = bk * P
                    for rk2 in range(NR // 2):
                        # fp8 exp'd scores for both ranks of the pair; PV
                        # contracts 256 keys per pass via DoubleRow
                        pbf = att.tile(
                            [P, 2, 2, 512], F8, name="pbf", tag="pbf", bufs=4
                        )
                        for i, rk in enumerate((2 * rk2, 2 * rk2 + 1)):
                            pss = psS.tile([P, 2, 512], F32, name="pss", tag="pss")
                            for sub in range(2):
                                po = sub * 64
                                nc.tensor.matmul(
                                    pss[:, sub, qo:],
                                    kfull[po:po + 64, hp, rk, bk * P:(bk + 1) * P],
                                    qT[po:po + 64, hp, qo:],
                                    start=True,
                                    stop=True,
                                    tile_position=(po, 0),
                                )
                            nc.scalar.activation(
                                pbf[:, i, :, qo:], pss[:, :, qo:], AF.Exp,
                                scale=1.0 / 8.0,
                            )
                            for sub in range(2):
                                nc.vector.tensor_mul(
                                    pbf[:, i, sub, qo:qo + P],
                                    pbf[:, i, sub, qo:qo + P],
                                    masks_sb[:, rk, :],
                                )
                        for sub in range(2):
                            h = 2 * hp + sub
                            nc.tensor.matmul(
                                psy[sub][:, qo:],
                                vfull[:, 2 * rk2:2 * rk2 + 2, bk, h * 65:(h + 1) * 65],
                                pbf[:, :, sub, qo:],
                                start=(bk == 0 and rk2 == 0),
                                stop=(bk == NBQ - 1 and rk2 == NR // 2 - 1),
                                perf_mode=DR,
                                skip_group_check=True,
                            )
                for sub in range(2):
                    po = sub * 64
                    recip = att.tile([1, 512], BF16, name="recip", tag="recip", bufs=4)
                    with nc.allow_low_precision(reason="softmax normalizer"):
                        nc.vector.reciprocal(recip[:, :], psy[sub][64:65, :])
                    bcast = att.tile([64, 512], BF16, name="bcast", tag="bcast", bufs=4)
                    nc.gpsimd.partition_broadcast(bcast[:, :], recip[0:1, :])
                    nc.vector.tensor_mul(
                        yT[po:po + 64, hp, :], psy[sub][0:64, :], bcast[:, :]
                    )

            psY.release()
            psS.release()
            att.release()
            attnspan.release()

            # ---------------- attention out-proj + residual ----------------
            psW = tc.alloc_tile_pool(name=f"psW{sfx}", bufs=3, space="PSUM")
            for bq in range(NBQ if level >= 4 else 0):
                xw = xts[bq]
                for cb in range(2):
                    ps = psW.tile([P, 512], F32, name="ps_w", tag="ps_w")
                    for c2 in range(CH // 2):
                        nc.tensor.matmul(
                            ps[:, :],
                            yT[:, 2 * c2:2 * c2 + 2, bq * P:(bq + 1) * P],
                            wo_sb[:, 2 * c2:2 * c2 + 2, cb * 512:(cb + 1) * 512],
                            start=(c2 == 0),
                            stop=(zb and c2 == CH // 2 - 1),
                            perf_mode=DR,
                        )
                    if not zb:
                        nc.tensor.matmul(
                            ps[:, :],
                            ones_sb[0:1, 0:P],
                            bo_sb[0:1, cb * 512:(cb + 1) * 512],
                            start=False,
                            stop=True,
                        )
                    nc.vector.scalar_tensor_tensor(
                        x2[:, bq, cb * 512:(cb + 1) * 512], ps[:, :],
                        scales_sb[:, 1:2], xw[:, cb * 512:(cb + 1) * 512],
                        OP.mult, OP.add,
                    )
            psW.release()
            wospan.release()
            xres.release()

            # ---------------- MLP ----------------
            mpool = tc.alloc_tile_pool(name=f"mpool{sfx}", bufs=1, side="right")
            mw = tc.alloc_tile_pool(name=f"mw{sfx}", bufs=2, side="right")
            h2T = mpool.tile([P, CH, 512], BF16, name="h2T")
            gT = mpool.tile([P, FCH, 512], BF16, name="gT")

            psT2 = tc.alloc_tile_pool(name=f"psT2{sfx}", bufs=4, space="PSUM")
            stats2 = [
                _ln_stats(nc, mw, x2[:, bq, :], f"l2_{bq}")
                for bq in range(NBQ if level >= 5 else 0)
            ]
            for bq in range(NBQ if level >= 5 else 0):
                z2 = mw.tile([P, C], BF16, name="z2", tag="z2")
                _ln_finalize(
                    nc, mw, x2[:, bq, :], z2[:, :], *stats2[bq], f"l2_{bq}"
                )
                for cg in range(CH // 4):
                    pt2 = psT2.tile([P, 4, P], BF16, name="pt2", tag="pt2")
                    for cc in range(4):
                        c = cg * 4 + cc
                        nc.tensor.transpose(
                            pt2[:, cc, :], z2[:, c * P:(c + 1) * P], ident_sb[:, :]
                        )
                    nc.scalar.activation(
                        h2T[:, cg * 4:(cg + 1) * 4, bq * P:(bq + 1) * P],
                        pt2[:, :, :], AF.Copy,
                    )
            psT2.release()

            psFC = tc.alloc_tile_pool(name=f"psFC{sfx}", bufs=3, space="PSUM")
            for ft in range(FCH if level >= 5 else 0):
                ps = psFC.tile([P, 512], F32, name="ps_fc", tag="ps_fc")
                for c in range(CH):
                    nc.tensor.matmul(
                        ps[:, :],
                        wfc_sb[:, c, ft * P:(ft + 1) * P],
                        h2T[:, c, :],
                        start=(c == 0),
                        stop=(c == CH - 1),
                    )
                nc.scalar.activation(
                    gT[:, ft, :], ps[:, :], AF.Gelu, bias=bfc_sb[:, ft:ft + 1]
                )
            psFC.release()

            psPJ = tc.alloc_tile_pool(name=f"psPJ{sfx}", bufs=1, space="PSUM")
            pres = [
                psPJ.tile([P, 512], F32, name=f"pres_{i}", tag=f"pres_{i}")
                for i in range(2 * NBQ)
            ] if level >= 5 else []
            for fc in range(FCH if level >= 5 else 0):
                wp = mw.tile([P, C], BF16, name="wp", tag="wp", bufs=6)
                nc.sync.dma_start(wp[:, :], wproj_in[fc])
                for bq in range(NBQ):
                    for cb in range(2):
                        nc.tensor.matmul(
                            pres[bq * 2 + cb][:, :],
                            gT[:, fc, bq * P:(bq + 1) * P],
                            wp[:, cb * 512:(cb + 1) * 512],
                            start=(fc == 0),
                            stop=(zb and fc == FCH - 1),
                        )
            for bq in range(NBQ if level >= 5 else 0):
                if not zb:
                    for cb in range(2):
                        nc.tensor.matmul(
                            pres[bq * 2 + cb][:, :],
                            ones_sb[0:1, 0:P],
                            bproj_sb[0:1, cb * 512:(cb + 1) * 512],
                            start=False,
                            stop=True,
                        )
                outt = mw.tile([P, C], F32, name="outt", tag="outt")
                for cb in range(2):
                    nc.vector.tensor_add(
                        outt[:, cb * 512:(cb + 1) * 512],
                        pres[bq * 2 + cb][:, :],
                        x2[:, bq, cb * 512:(cb + 1) * 512],
                    )
                    nc.sync.dma_start(
                        out_tgt[bq][:, cb * 512:(cb + 1) * 512],
                        outt[:, cb * 512:(cb + 1) * 512],
                    )
            psPJ.release()
            mw.release()
            mpool.release()
            wfcpool.release()
            x2pool.release()
            persist.release()
            if level < 5 and reps > 1:
                # disabled-phase timing builds lack the full data chain;
                # serialize reps explicitly
                tc.strict_bb_all_engine_barrier()
        dramp.release()

    if not sim:
        nc.compile()
    _CACHE[(level, reps, sim, zb)] = nc
    return nc


def prepare_in_maps(inputs):
    """Host-side prep: fold LN, cast/shard weights, build per-core input maps."""
    x = np.asarray(inputs["x"], dtype=np.float32)
    ln1_w = np.asarray(inputs["ln1_w"], dtype=np.float32)
    ln1_b = np.asarray(inputs["ln1_b"], dtype=np.float32)
    Wqkv = np.asarray(inputs["Wqkv"], dtype=np.float32)
    bqkv = np.asarray(inputs["bqkv"], dtype=np.float32)
    Wo = np.asarray(inputs["Wo"], dtype=np.float32)
    bo = np.asarray(inputs["bo"], dtype=np.float32)
    ln2_w = np.asarray(inputs["ln2_w"], dtype=np.float32)
    ln2_b = np.asarray(inputs["ln2_b"], dtype=np.float32)
    Wfc = np.asarray(inputs["Wfc"], dtype=np.float32)
    bfc = np.asarray(inputs["bfc"], dtype=np.float32)
    Wproj = np.asarray(inputs["Wproj"], dtype=np.float32)
    bproj = np.asarray(inputs["bproj"], dtype=np.float32)

    # Fold LN affine params into the downstream matmuls.
    Wqkv_f = ln1_w[:, None] * Wqkv
    bqkv_f = bqkv + ln1_b @ Wqkv
    Wfc_f = ln2_w[:, None] * Wfc
    bfc_f = bfc + ln2_b @ Wfc

    # fp8 (DoubleRow) for QKV and Wo; dequant scales folded into epilogues.
    s_qkv = float(np.abs(Wqkv_f).max()) / 224.0
    s_o = float(np.abs(Wo).max()) / 224.0
    wqkv_h = np.ascontiguousarray(
        (Wqkv_f / s_qkv).astype(F8_NP).reshape(CH, P, 3 * C)
    )
    wo_h = np.ascontiguousarray((Wo / s_o).astype(F8_NP).reshape(CH, P, C))
    wfc_h = np.ascontiguousarray(Wfc_f.astype(BF16_NP).reshape(CH, P, FF))
    wproj_h = np.ascontiguousarray(Wproj.astype(BF16_NP).reshape(FCH, P, C))
    bqk_h = np.ascontiguousarray(bqkv_f[: 2 * C].reshape(2 * CH, P))
    # bias for V is added into the pre-dequant PSUM, so pre-divide by s_qkv
    bv_h = np.ascontiguousarray(
        (bqkv_f[2 * C:] / s_qkv).astype(BF16_NP).reshape(1, C)
    )
    bo_h = (bo / s_o).astype(BF16_NP).reshape(1, C)
    scales_h = np.tile(np.array([[s_qkv, s_o]], np.float32), (P, 1))
    bfc_h = np.ascontiguousarray(bfc_f.reshape(FCH, P))
    bproj_h = bproj.astype(BF16_NP).reshape(1, C)
    ident_h = np.eye(P, dtype=BF16_NP)
    ones_h = np.ones((1, P), BF16_NP)
    kk = np.arange(P)[:, None]
    qq = np.arange(P)[None, :]
    tri_incl = (kk <= qq).astype(F8_NP)
    tri_strict = (kk < qq).astype(F8_NP)

    in_maps = []
    for core in range(8):
        b, s = divmod(core, 4)
        x_loc = np.ascontiguousarray(x[b, s::4, :])
        x_c = x_loc.reshape(NBQ, P, C)
        xT_c = np.ascontiguousarray(x_loc.T.astype(BF16_NP).reshape(CH, P, 512))
        masks_h = np.stack(
            [tri_incl if rk <= s else tri_strict for rk in range(NR)]
        )
        in_maps.append(
            {
                "x_c": x_c,
                "xT_c": xT_c,
                "wqkv": wqkv_h,
                "wo": wo_h,
                "wfc": wfc_h,
                "wproj": wproj_h,
                "bqk": bqk_h,
                "bv": bv_h,
                "bo_r": bo_h,
                "bfc_r": bfc_h,
                "bproj_r": bproj_h,
                "ident": ident_h,
                "scales": scales_h,
                "ones_r": ones_h,
                "masks": masks_h,
            }
        )

    return in_maps


def assemble_output(results):
    out = np.empty((B, T, C), np.float32)
    for core in range(8):
        b, s = divmod(core, 4)
        out[b, s::4, :] = results[core]["out_c"].reshape(NR * P, C)
    return out


def kernel(**inputs):
    global LAST_RESULTS
    in_maps = prepare_in_maps(inputs)
    zb = not (
        np.any(np.asarray(inputs["bqkv"])) or np.any(np.asarray(inputs["bo"]))
        or np.any(np.asarray(inputs["bproj"]))
    )
    nc = _build(LEVEL, REPS, zb=zb)
    res = run_bass_kernel_spmd(
        nc, in_maps, core_ids=list(range(8)), trace=TRACE
    )
    LAST_RESULTS = res
    return assemble_output(res.results)

